# revision 1
# baseline (speedup 1.0000x reference)
"""Trainium2 Bass kernel for windowed mean-pooling (segment_reduce).

Computes, for each (batch b, window w):
    out[b, w, :] = mean over t in [begins[b,w], ends'[b,w]) of features[b, t, :]
where ends' = clip(ends, begins, begins + 8) (the reference gathers at most
MAX_WINDOW=8 tokens) and empty windows produce 0 (count clamped to >= 1).

Strategy (data-parallel over batch, one sample per NeuronCore):
  - The kernel is HBM-bound, so input bytes are minimized: features ship as
    fp16 (6.3 MB instead of 12.6 MB fp32; ~2e-4 rel err on the windowed
    means, and fp16 matmuls run at full PE rate unlike fp32 which lowers
    to two HW passes); begins/ends arrive as one 8 KB fp16 row (values
    shifted by -2048 so 0..4096 are fp16-exact) and are broadcast across
    partitions on-chip with K=1 fp16 ones-matmuls on the idle PE.
  - Slab layout in SBUF: token t on partition (t % 128), K-tile (t // 128).
  - For each 128-window output block: out_block = S^T @ F on the
    TensorEngine, where S[t, w] = (begins[w] <= t < ends[w]) is built per
    K-tile by the VectorEngine from the broadcast rows with fused
    compare ops (S in fp16: 0/1 exact). Accumulate over the block's
    K-tiles in PSUM, scale rows by 1/count on the ScalarEngine
    (activation Copy with per-partition scale), DMA out.
  - Per-block K-tile ranges come from the host (actual index data), taking
    the union across the 8 cores so one SPMD program serves all cores
    (masks are zero outside a core's true range -> contributes nothing).
  - DMA assignment: features via GPSIMD SWDGE (descriptor generation off
    the critical sequencers, small chunks first so the PE starts early),
    metadata on SP, outputs on ACT.
"""

import os
import sys

import numpy as np

for _p in ("/opt/trn_rl_repo", "/root/.axon_site/_ro/trn_rl_repo"):
    if os.path.isdir(_p) and _p not in sys.path:
        sys.path.insert(0, _p)

from concourse import bacc, mybir  # noqa: E402
import concourse.tile as tile  # noqa: E402
from concourse.bass_utils import run_bass_kernel_spmd  # noqa: E402

B, T, D, W = 8, 4096, 768, 2048
MAXWIN = 8
P = 128
NBLK = W // P  # 16 window blocks of 128 windows
NKT = T // P  # 32 K-tiles of 128 tokens
FCHUNKS = (1, 1, 2, 4, 4, 4, 4, 4, 4, 2, 1, 1)  # K-tiles per feature DMA chunk
MCH = 512  # windows per metadata DMA chunk
F32 = mybir.dt.float32
FP16 = mybir.dt.float16
I16 = mybir.dt.int16


def _build_program(klo, khi):
    """Build the SPMD Bass program given per-block K-tile ranges [klo, khi)."""
    nc = bacc.Bacc(None)

    fhi_d = nc.declare_dram_parameter("fhi", [P, NKT, D], FP16, isOutput=False)
    meta = nc.declare_dram_parameter("meta", [1, 2, W], FP16, isOutput=False)
    ioiv = nc.declare_dram_parameter("ioiv", [P, P], F32, isOutput=False)
    out_d = nc.declare_dram_parameter("out", [W, D], F32, isOutput=True)

    # token t = n*128 + p -> fhi[p, n, d] (host-shuffled for contiguous
    # per-partition DMA descriptors); window w = i*128 + p -> [p, i, d]
    fhi_r = fhi_d[:]
    out_r = out_d[:].rearrange("(n p) d -> p n d", p=P)

    # For each K-tile, the contiguous span of blocks that consume it.
    strip_rng = {}
    for k in range(NKT):
        blks = [i for i in range(NBLK) if klo[i] <= k < khi[i]]
        if blks:
            strip_rng[k] = (min(blks), max(blks) + 1)

    with tile.TileContext(nc) as tc:
        with (
            tc.tile_pool(name="metap", bufs=1) as meta_pool,
            tc.tile_pool(name="fslab", bufs=1) as f_pool,
            tc.tile_pool(name="m2p", bufs=4) as m2_pool,
            tc.tile_pool(name="maskp", bufs=12) as mask_pool,
            tc.tile_pool(name="outp", bufs=8) as out_pool,
            tc.tile_pool(name="psum", bufs=4, space="PSUM") as psum_pool,
        ):
            # iota [P, :NKT] (iota[p, k] = 128k + p), 1/count [P, NKT:NKT+NBLK],
            # zero-padded to [P, 128] so DMA descriptors stay >= 512 B.
            ioiv_sb = meta_pool.tile([P, P], F32)
            nc.sync.dma_start(out=ioiv_sb[:], in_=ioiv[:])
            io_sb = ioiv_sb[:, 0:NKT]
            iv_sb = ioiv_sb[:, NKT : NKT + NBLK]

            # begins/ends arrive as ONE 8 KB fp16 row (values shifted by
            # -2048 so 0..4096 are all fp16-exact) and are broadcast across
            # the 128 partitions with K=1 fp16 ones-matmuls on the idle PE,
            # saving ~1 MB of HBM traffic.
            rows_sb = meta_pool.tile([1, 2, W], FP16)
            nc.sync.dma_start(out=rows_sb[:], in_=meta[:])
            ones_sb = meta_pool.tile([1, P], FP16)
            nc.vector.memset(ones_sb[:], 1.0)
            be_sb = meta_pool.tile([P, 2, W], FP16)
            for h in range(2):
                for s in range(W // MCH):
                    sl = slice(s * MCH, (s + 1) * MCH)
                    pb = psum_pool.tile([P, MCH], F32, name=f"pb{h}_{s}", tag="ps")
                    nc.tensor.matmul(
                        pb[:], ones_sb[:], rows_sb[:, h, sl], start=True, stop=True
                    )
                    nc.vector.tensor_copy(out=be_sb[:, h, sl], in_=pb[:])

            # Feature slab chunks (fp16), small chunks first.
            fhi_tiles = []
            k2chunk = []
            k0 = 0
            for j, sz in enumerate(FCHUNKS):
                fh = f_pool.tile([P, sz, D], FP16, name=f"fh{j}", tag=f"fh{j}")
                nc.gpsimd.dma_start(out=fh[:], in_=fhi_r[:, k0 : k0 + sz, :])
                fhi_tiles.append(fh)
                for s in range(sz):
                    k2chunk.append((j, s))
                k0 += sz
            assert k0 == NKT

            # Per-K-tile mask strips over the span of blocks that use them,
            # in [token, window] layout: mask[p, w] = (b[w] <= t) * (e[w] > t)
            # with t = 128k + p.
            masks = {}
            for k in sorted(strip_rng):
                blo, bhi = strip_rng[k]
                wlo, whi = blo * P, bhi * P
                wn = whi - wlo
                m2 = m2_pool.tile([P, wn], FP16, name=f"m2_{k}", tag="m2")
                msk = mask_pool.tile([P, wn], FP16, name=f"mask_{k}", tag="mask")
                nc.vector.tensor_scalar(
                    m2[:], be_sb[:, 1, wlo:whi], io_sb[:, k : k + 1], None,
                    mybir.AluOpType.is_gt,
                )
                nc.vector.scalar_tensor_tensor(
                    msk[:], be_sb[:, 0, wlo:whi], io_sb[:, k : k + 1], m2[:],
                    mybir.AluOpType.is_le, mybir.AluOpType.mult,
                )
                masks[k] = (msk, blo)

            for i in range(NBLK):
                ps = psum_pool.tile([P, D], F32, name=f"ps{i}", tag="ps")
                for k in range(klo[i], khi[i]):
                    msk, blo = masks[k]
                    lh = msk[:, (i - blo) * P : (i - blo + 1) * P]
                    cj, cs = k2chunk[k]
                    rh = fhi_tiles[cj][:, cs, :]
                    first = k == klo[i]
                    last = k == khi[i] - 1
                    for n0, nn in ((0, 512), (512, 256)):
                        nc.tensor.matmul(
                            ps[:, n0 : n0 + nn], lh, rh[:, n0 : n0 + nn],
                            start=first, stop=(last and n0 == 512),
                        )
                os = out_pool.tile([P, D], F32, name=f"os{i}", tag="os")
                nc.scalar.mul(out=os[:], in_=ps[:], mul=iv_sb[:, i : i + 1])
                # Outputs on the SP ring (idle after metadata) so the ACT
                # sequencer never stalls between evacuation copies.
                nc.sync.dma_start(out=out_r[:, i, :], in_=os[:])

    nc.finalize()
    return nc


def _prepare(features, begins, ends):
    feats = np.asarray(features, dtype=np.float32)
    assert feats.shape == (B, T, D), feats.shape
    b = np.clip(np.asarray(begins).astype(np.int64), 0, T - 1)
    e = np.asarray(ends).astype(np.int64)
    # Reference gathers at most MAXWIN tokens starting at b; empty -> count 1.
    e_eff = np.clip(e, b, np.minimum(b + MAXWIN, T))
    counts = np.maximum(e_eff - b, 1).astype(np.float32)
    inv = (1.0 / counts).astype(np.float32)

    bw = b.reshape(B, NBLK, P)
    ew = e_eff.reshape(B, NBLK, P)
    klo_pc = bw.min(-1) // P  # [B, NBLK]
    khi_pc = (np.maximum(ew.max(-1) - 1, bw.min(-1)) // P) + 1
    klo = klo_pc.min(0).astype(int)
    khi = khi_pc.max(0).astype(int)
    khi = np.minimum(np.maximum(khi, klo + 1), NKT)

    # shuffle to [P, NKT, D]: partition p holds tokens {p, 128+p, ...}
    hi = np.ascontiguousarray(
        feats.astype(np.float16).reshape(B, NKT, P, D).transpose(0, 2, 1, 3)
    )

    iota = (
        np.arange(NKT)[None, :] * P + np.arange(P)[:, None] - 2048
    ).astype(np.float32)
    in_maps = []
    for c in range(B):
        metac = np.ascontiguousarray(
            (np.stack([b[c], e_eff[c]]) - 2048).astype(np.float16).reshape(1, 2, W)
        )
        ioiv = np.zeros((P, P), np.float32)
        ioiv[:, 0:NKT] = iota
        ioiv[:, NKT : NKT + NBLK] = inv[c].reshape(NBLK, P).T
        in_maps.append(
            {
                "fhi": hi[c],
                "meta": metac,
                "ioiv": ioiv,
            }
        )
    return list(klo), list(khi), in_maps


def run(features, begins, ends, trace=False):
    """Build + run on 8 NeuronCores; returns (output, BassKernelResults)."""
    klo, khi, in_maps = _prepare(features, begins, ends)
    nc = _build_program(klo, khi)
    res = run_bass_kernel_spmd(nc, in_maps, list(range(B)), trace=trace)
    out = np.stack([res.results[c]["out"] for c in range(B)], axis=0)
    return out, res


def kernel(features, begins, ends):
    out, _ = run(features, begins, ends, trace=False)
    return out



# revision 2
# speedup vs baseline: 1.0541x; 1.0541x over previous
"""Trainium2 Bass kernel for windowed mean-pooling (segment_reduce).

Computes, for each (batch b, window w):
    out[b, w, :] = mean over t in [begins[b,w], ends'[b,w]) of features[b, t, :]
where ends' = clip(ends, begins, begins + 8) (the reference gathers at most
MAX_WINDOW=8 tokens) and empty windows produce 0 (count clamped to >= 1).

Strategy (data-parallel over batch, one sample per NeuronCore):
  - The kernel was TensorEngine-bound (masked-matmul columns at fp16 run
    1 cycle/row).  Features now ship as TWO fp8e4m3 planes A + R with
    A = fp8(x), R = fp8(x - A)  (combined rel err ~8e-4), and each
    (block, K-tile) product runs ONE DoubleRow fp8 matmul contracting both
    planes at 0.5 cycles/row -- half the PE time of the fp16 version for
    the same 2 bytes/element of HBM traffic.
  - The 0/1 masks are built in fp8 (exact) by the VectorEngine from
    broadcast begin/end rows; the DoubleRow lhsT reads the same mask for
    both planes via a stride-0 middle AP dim (no duplication).
  - Output is written fp16 (host upcasts to fp32; ~3e-4 rel err), halving
    DMA-out bytes.  PSUM rows are scaled by 1/count on the ScalarEngine
    (activation mul with per-partition scale) straight into fp16.
  - begins/ends arrive as ONE 8 KB fp16 row (shifted by -2048 so 0..4096
    are fp16-exact), broadcast across partitions with K=1 fp16
    ones-matmuls on the PE; evacuation PSUM->SBUF runs on the ScalarEngine
    (the VectorEngine is loaded with mask builds).
  - Slab layout in SBUF: token t on partition (t % 128), K-tile (t // 128),
    planes [A; R] contiguous per K-tile.
  - Per-block K-tile ranges come from the host (actual index data), taking
    the union across the 8 cores so one SPMD program serves all cores
    (masks are zero outside a core's true range -> contributes nothing).
  - DMA assignment: features via GPSIMD SWDGE (descriptor generation off
    the critical sequencers, small chunks first so the PE starts early),
    metadata + outputs on the SP HWDGE ring.
"""

import os
import sys

import numpy as np

for _p in ("/opt/trn_rl_repo", "/root/.axon_site/_ro/trn_rl_repo"):
    if os.path.isdir(_p) and _p not in sys.path:
        sys.path.insert(0, _p)

from concourse import bacc, mybir  # noqa: E402
import concourse.tile as tile  # noqa: E402
from concourse.bass_utils import run_bass_kernel_spmd  # noqa: E402

B, T, D, W = 8, 4096, 768, 2048
MAXWIN = 8
P = 128
NBLK = W // P  # 16 window blocks of 128 windows
NKT = T // P  # 32 K-tiles of 128 tokens
FCHUNKS = (1, 1, 2, 4, 4, 4, 4, 4, 4, 2, 1, 1)  # K-tiles per feature DMA chunk
MCH = 512  # windows per metadata DMA chunk
F32 = mybir.dt.float32
FP16 = mybir.dt.float16
FP8 = mybir.dt.float8e4
DR = mybir.MatmulPerfMode.DoubleRow


def _build_program(klo, khi):
    """Build the SPMD Bass program given per-block K-tile ranges [klo, khi)."""
    nc = bacc.Bacc(None)

    fhi_d = nc.declare_dram_parameter("fhi", [P, NKT, 2, D], FP8, isOutput=False)
    meta = nc.declare_dram_parameter("meta", [1, 2, W], FP16, isOutput=False)
    ioiv = nc.declare_dram_parameter("ioiv", [P, P], F32, isOutput=False)
    out_d = nc.declare_dram_parameter("out", [W, D], FP16, isOutput=True)

    # token t = n*128 + p -> fhi[p, n, plane, d] (host-shuffled for contiguous
    # per-partition DMA descriptors); window w = i*128 + p -> [p, i, d]
    fhi_r = fhi_d[:]
    out_r = out_d[:].rearrange("(n p) d -> p n d", p=P)

    # For each K-tile, the contiguous span of blocks that consume it.
    strip_rng = {}
    for k in range(NKT):
        blks = [i for i in range(NBLK) if klo[i] <= k < khi[i]]
        if blks:
            strip_rng[k] = (min(blks), max(blks) + 1)

    with tile.TileContext(nc) as tc:
        with (
            tc.tile_pool(name="metap", bufs=1) as meta_pool,
            tc.tile_pool(name="fslab", bufs=1) as f_pool,
            tc.tile_pool(name="m2p", bufs=4) as m2_pool,
            tc.tile_pool(name="maskp", bufs=12) as mask_pool,
            tc.tile_pool(name="outp", bufs=8) as out_pool,
            tc.tile_pool(name="psum", bufs=4, space="PSUM") as psum_pool,
        ):
            # iota [P, :NKT] (iota[p, k] = 128k + p), 1/count [P, NKT:NKT+NBLK],
            # zero-padded to [P, 128] so DMA descriptors stay >= 512 B.
            ioiv_sb = meta_pool.tile([P, P], F32)
            nc.sync.dma_start(out=ioiv_sb[:], in_=ioiv[:])
            io_sb = ioiv_sb[:, 0:NKT]
            iv_sb = ioiv_sb[:, NKT : NKT + NBLK]

            # begins/ends arrive as ONE 8 KB fp16 row (values shifted by
            # -2048 so 0..4096 are all fp16-exact) and are broadcast across
            # the 128 partitions with K=1 fp16 ones-matmuls on the idle PE,
            # saving ~1 MB of HBM traffic.
            rows_sb = meta_pool.tile([1, 2, W], FP16)
            nc.sync.dma_start(out=rows_sb[:], in_=meta[:])
            ones_sb = meta_pool.tile([1, P], FP16)
            nc.vector.memset(ones_sb[:], 1.0)
            be_sb = meta_pool.tile([P, 2, W], FP16)
            for h in range(2):
                for s in range(W // MCH):
                    sl = slice(s * MCH, (s + 1) * MCH)
                    pb = psum_pool.tile([P, MCH], F32, name=f"pb{h}_{s}", tag="ps")
                    nc.tensor.matmul(
                        pb[:], ones_sb[:], rows_sb[:, h, sl], start=True, stop=True
                    )
                    # Evacuate on the ScalarEngine (activation copy); the
                    # VectorEngine is saturated with mask builds.
                    nc.scalar.mul(out=be_sb[:, h, sl], in_=pb[:], mul=1.0)

            # Feature slab chunks (fp8 planes A,R per K-tile), small first.
            fhi_tiles = []
            k2chunk = []
            k0 = 0
            for j, sz in enumerate(FCHUNKS):
                fh = f_pool.tile([P, sz, 2, D], FP8, name=f"fh{j}", tag=f"fh{j}")
                nc.gpsimd.dma_start(out=fh[:], in_=fhi_r[:, k0 : k0 + sz, :, :])
                fhi_tiles.append(fh)
                for s in range(sz):
                    k2chunk.append((j, s))
                k0 += sz
            assert k0 == NKT

            # Per-K-tile mask strips over the span of blocks that use them,
            # in [token, window] layout: mask[p, w] = (b[w] <= t) * (e[w] > t)
            # with t = 128k + p.  fp8 (0/1 exact) for the DoubleRow matmul.
            masks = {}
            for k in sorted(strip_rng):
                blo, bhi = strip_rng[k]
                wlo, whi = blo * P, bhi * P
                wn = whi - wlo
                m2 = m2_pool.tile([P, wn], FP16, name=f"m2_{k}", tag="m2")
                msk = mask_pool.tile([P, wn], FP8, name=f"mask_{k}", tag="mask")
                nc.vector.tensor_scalar(
                    m2[:], be_sb[:, 1, wlo:whi], io_sb[:, k : k + 1], None,
                    mybir.AluOpType.is_gt,
                )
                nc.vector.scalar_tensor_tensor(
                    msk[:], be_sb[:, 0, wlo:whi], io_sb[:, k : k + 1], m2[:],
                    mybir.AluOpType.is_le, mybir.AluOpType.mult,
                )
                masks[k] = (msk, blo)

            for i in range(NBLK):
                ps = psum_pool.tile([P, D], F32, name=f"ps{i}", tag="ps")
                for k in range(klo[i], khi[i]):
                    msk, blo = masks[k]
                    # Same 0/1 mask feeds both DoubleRow planes via a
                    # stride-0 middle AP dim.
                    lh = (
                        msk[:, (i - blo) * P : (i - blo + 1) * P]
                        .unsqueeze(1)
                        .broadcast_to((P, 2, P))
                    )
                    cj, cs = k2chunk[k]
                    rh = fhi_tiles[cj]
                    first = k == klo[i]
                    last = k == khi[i] - 1
                    for n0, nn in ((0, 512), (512, 256)):
                        nc.tensor.matmul(
                            ps[:, n0 : n0 + nn],
                            lh,
                            rh[:, cs, :, n0 : n0 + nn],
                            start=first,
                            stop=last,
                            perf_mode=DR,
                        )
                os = out_pool.tile([P, D], FP16, name=f"os{i}", tag="os")
                nc.scalar.mul(out=os[:], in_=ps[:], mul=iv_sb[:, i : i + 1])
                # Outputs on the SP ring (idle after metadata) so the ACT
                # sequencer never stalls between evacuation copies.
                nc.sync.dma_start(out=out_r[:, i, :], in_=os[:])

    nc.finalize()
    return nc


def _prepare(features, begins, ends):
    feats = np.asarray(features, dtype=np.float32)
    assert feats.shape == (B, T, D), feats.shape
    b = np.clip(np.asarray(begins).astype(np.int64), 0, T - 1)
    e = np.asarray(ends).astype(np.int64)
    # Reference gathers at most MAXWIN tokens starting at b; empty -> count 1.
    e_eff = np.clip(e, b, np.minimum(b + MAXWIN, T))
    counts = np.maximum(e_eff - b, 1).astype(np.float32)
    inv = (1.0 / counts).astype(np.float32)

    bw = b.reshape(B, NBLK, P)
    ew = e_eff.reshape(B, NBLK, P)
    klo_pc = bw.min(-1) // P  # [B, NBLK]
    khi_pc = (np.maximum(ew.max(-1) - 1, bw.min(-1)) // P) + 1
    klo = klo_pc.min(0).astype(int)
    khi = khi_pc.max(0).astype(int)
    khi = np.minimum(np.maximum(khi, klo + 1), NKT)

    # fp8 A/R planes: A = fp8(x), R = fp8(x - A); shuffle to [P, NKT, 2, D]
    # (partition p holds tokens {p, 128+p, ...}, planes contiguous per tile)
    f8 = mybir.dt.np(FP8)
    A = feats.astype(f8)
    R = (feats - A.astype(np.float32)).astype(f8)
    hi = np.ascontiguousarray(
        np.stack(
            [A.reshape(B, NKT, P, D), R.reshape(B, NKT, P, D)], axis=3
        ).transpose(0, 2, 1, 3, 4)
    )  # [B, P, NKT, 2, D]

    iota = (
        np.arange(NKT)[None, :] * P + np.arange(P)[:, None] - 2048
    ).astype(np.float32)
    in_maps = []
    for c in range(B):
        metac = np.ascontiguousarray(
            (np.stack([b[c], e_eff[c]]) - 2048).astype(np.float16).reshape(1, 2, W)
        )
        ioiv = np.zeros((P, P), np.float32)
        ioiv[:, 0:NKT] = iota
        ioiv[:, NKT : NKT + NBLK] = inv[c].reshape(NBLK, P).T
        in_maps.append(
            {
                "fhi": hi[c],
                "meta": metac,
                "ioiv": ioiv,
            }
        )
    return list(klo), list(khi), in_maps


def run(features, begins, ends, trace=False):
    """Build + run on 8 NeuronCores; returns (output, BassKernelResults)."""
    klo, khi, in_maps = _prepare(features, begins, ends)
    nc = _build_program(klo, khi)
    res = run_bass_kernel_spmd(nc, in_maps, list(range(B)), trace=trace)
    out = np.stack(
        [res.results[c]["out"].astype(np.float32) for c in range(B)], axis=0
    )
    return out, res


def kernel(features, begins, ends):
    out, _ = run(features, begins, ends, trace=False)
    return out


# revision 3
# speedup vs baseline: 1.0566x; 1.0024x over previous
"""Trainium2 Bass kernel for windowed mean-pooling (segment_reduce).

Computes, for each (batch b, window w):
    out[b, w, :] = mean over t in [begins[b,w], ends'[b,w]) of features[b, t, :]
where ends' = clip(ends, begins, begins + 8) (the reference gathers at most
MAX_WINDOW=8 tokens) and empty windows produce 0 (count clamped to >= 1).

Strategy (data-parallel over batch, one sample per NeuronCore):
  - The kernel is TensorEngine-bound: a masked matmul out_blk = S^T @ F per
    (window-block, token-K-tile) costs ~165 ns fixed + 0.42 ns/column, so
    total PE time ~ Sigma * 651 ns where Sigma = sum of per-block K-tile
    range lengths.  Two levers applied:
      1. fp8 DoubleRow: features ship as TWO fp8e4m3 planes A + R with
         A = fp8(x), R = fp8(x - A) (combined rel err ~8e-4); each matmul
         contracts both planes (K=256) in the same time fp16 contracts 128.
      2. Window permutation: windows are reassigned to ~17 partially-filled
         blocks with data-adaptive tile ranges (pad slots get null masks),
         shrinking Sigma from 62 (sorted blocks, 8-core union) to ~50,
         which is near the combinatorial floor 31 + n_blocks.
  - The 0/1 masks are built in fp8 (exact) by the VectorEngine from
    broadcast begin/end rows; the DoubleRow lhsT reads the same mask for
    both planes via a stride-0 middle AP dim (no duplication).
  - begins/ends arrive as ONE fp16 row (shifted by -2048 so 0..4096 are
    fp16-exact) and are broadcast across the 128 partitions by a stride-0
    DRAM->SBUF DMA on the ACT HWDGE ring (frees the PE + evac engines).
  - Output is written fp16 (host upcasts to fp32; ~3e-4 rel err), halving
    DMA-out bytes.  PSUM rows are scaled by 1/count on the ScalarEngine
    (activation mul with per-partition scale) straight into fp16.
  - Slab layout in SBUF: token t on partition (t % 128), K-tile (t // 128),
    planes [A; R] contiguous per K-tile.
  - DMA assignment: features via GPSIMD SWDGE (descriptor generation off
    the critical sequencers, small chunks first so the PE starts early),
    metadata broadcast on ACT, ioiv + outputs on the SP HWDGE ring.
"""

import os
import sys

import numpy as np

for _p in ("/opt/trn_rl_repo", "/root/.axon_site/_ro/trn_rl_repo"):
    if os.path.isdir(_p) and _p not in sys.path:
        sys.path.insert(0, _p)

from concourse import bacc, mybir  # noqa: E402
import concourse.tile as tile  # noqa: E402
from concourse.bass_utils import run_bass_kernel_spmd  # noqa: E402

B, T, D, W = 8, 4096, 768, 2048
MAXWIN = 8
P = 128
NKT = T // P  # 32 K-tiles of 128 tokens
LSPAN = 3  # tiles per block range in the adaptive assignment
FCHUNKS = (1, 1, 2, 4, 4, 4, 4, 4, 4, 2, 1, 1)  # K-tiles per feature DMA chunk
BCH = 512  # windows per metadata broadcast DMA chunk
F32 = mybir.dt.float32
FP16 = mybir.dt.float16
FP8 = mybir.dt.float8e4
DR = mybir.MatmulPerfMode.DoubleRow


def _core_uv(b, e_eff):
    u = b // P
    v = np.maximum(e_eff - 1, b) // P
    return u, v


def _assign(b_all, ee_all):
    """Adaptive shared block ranges + per-core window permutation.

    Returns (ranges, perms): ranges = [(klo_j, khi_j)], perms[c][j] = list of
    128 window ids (-1 = pad).  Every window's span tiles fit its block's
    range; blocks may be partially filled.
    """
    nb_cores = b_all.shape[0]
    UV = [_core_uv(b_all[c], ee_all[c]) for c in range(nb_cores)]
    unass = [set(range(W)) for _ in range(nb_cores)]
    ranges = []
    perms = [[] for _ in range(nb_cores)]
    s_prev = None
    while any(unass):
        mins = [
            min(UV[c][0][w] for w in unass[c])
            for c in range(nb_cores)
            if unass[c]
        ]
        s = min(mins)
        if s_prev is not None:
            s = max(min(s, s_prev + (LSPAN - 1)), s_prev)
        hi = min(s + LSPAN, NKT)
        ranges.append([s, hi])
        for c in range(nb_cores):
            u, v = UV[c]
            elig = [w for w in unass[c] if u[w] >= s and v[w] < hi]
            elig.sort(key=lambda w: (v[w], u[w]))
            take = elig[:P]
            unass[c] -= set(take)
            perms[c].append(take + [-1] * (P - len(take)))
        s_prev = s
        assert len(ranges) <= 24, "assignment runaway"
    # tighten each block's range to the span actually used (any core)
    for j in range(len(ranges)):
        lo, hi = NKT, 0
        for c in range(nb_cores):
            u, v = UV[c]
            ws = [w for w in perms[c][j] if w >= 0]
            if ws:
                lo = min(lo, min(u[w] for w in ws))
                hi = max(hi, max(v[w] for w in ws) + 1)
        if lo < hi:
            ranges[j] = [lo, hi]
    return ranges, perms


def _build_program(ranges):
    """Build the SPMD Bass program for the given per-block K-tile ranges."""
    nb = len(ranges)
    Wp = nb * P
    nc = bacc.Bacc(None)

    fhi_d = nc.declare_dram_parameter("fhi", [P, NKT, 2, D], FP8, isOutput=False)
    meta = nc.declare_dram_parameter("meta", [1, 2, Wp], FP16, isOutput=False)
    ioiv = nc.declare_dram_parameter("ioiv", [P, P], F32, isOutput=False)
    out_d = nc.declare_dram_parameter("out", [Wp, D], FP16, isOutput=True)

    fhi_r = fhi_d[:]
    out_r = out_d[:].rearrange("(n p) d -> p n d", p=P)

    # For each K-tile, the contiguous span of blocks that consume it.
    strip_rng = {}
    for k in range(NKT):
        blks = [j for j in range(nb) if ranges[j][0] <= k < ranges[j][1]]
        if blks:
            strip_rng[k] = (min(blks), max(blks) + 1)

    with tile.TileContext(nc) as tc:
        with (
            tc.tile_pool(name="metap", bufs=1) as meta_pool,
            tc.tile_pool(name="fslab", bufs=1) as f_pool,
            tc.tile_pool(name="m2p", bufs=4) as m2_pool,
            tc.tile_pool(name="maskp", bufs=12) as mask_pool,
            tc.tile_pool(name="outp", bufs=8) as out_pool,
            tc.tile_pool(name="psum", bufs=4, space="PSUM") as psum_pool,
        ):
            # iota [P, :NKT] (iota[p, k] = 128k + p - 2048), 1/count
            # [P, NKT:NKT+nb], zero-padded to [P, 128].
            ioiv_sb = meta_pool.tile([P, P], F32)
            nc.sync.dma_start(out=ioiv_sb[:], in_=ioiv[:])
            io_sb = ioiv_sb[:, 0:NKT]
            iv_sb = ioiv_sb[:, NKT : NKT + nb]

            # begins/ends: ONE fp16 row (shifted -2048) broadcast across the
            # 128 partitions by stride-0 DRAM->SBUF DMAs, chunked so early
            # mask builds start before the whole row has landed.
            be_sb = meta_pool.tile([P, 2, Wp], FP16)
            for c0 in range(0, Wp, BCH):
                c1 = min(c0 + BCH, Wp)
                nc.scalar.dma_start(
                    out=be_sb[:, :, c0:c1],
                    in_=meta[:, :, c0:c1].broadcast_to((P, 2, c1 - c0)),
                )

            # Feature slab chunks (fp8 planes A,R per K-tile), small first.
            fhi_tiles = []
            k2chunk = []
            k0 = 0
            for j, sz in enumerate(FCHUNKS):
                fh = f_pool.tile([P, sz, 2, D], FP8, name=f"fh{j}", tag=f"fh{j}")
                nc.gpsimd.dma_start(out=fh[:], in_=fhi_r[:, k0 : k0 + sz, :, :])
                fhi_tiles.append(fh)
                for s in range(sz):
                    k2chunk.append((j, s))
                k0 += sz
            assert k0 == NKT

            # Per-K-tile mask strips over the span of blocks that use them,
            # in [token, window] layout: mask[p, w] = (b[w] <= t) * (e[w] > t)
            # with t = 128k + p.  fp8 (0/1 exact) for the DoubleRow matmul.
            masks = {}
            for k in sorted(strip_rng):
                blo, bhi = strip_rng[k]
                wlo, whi = blo * P, bhi * P
                wn = whi - wlo
                m2 = m2_pool.tile([P, wn], FP16, name=f"m2_{k}", tag="m2")
                msk = mask_pool.tile([P, wn], FP8, name=f"mask_{k}", tag="mask")
                nc.vector.tensor_scalar(
                    m2[:], be_sb[:, 1, wlo:whi], io_sb[:, k : k + 1], None,
                    mybir.AluOpType.is_gt,
                )
                nc.vector.scalar_tensor_tensor(
                    msk[:], be_sb[:, 0, wlo:whi], io_sb[:, k : k + 1], m2[:],
                    mybir.AluOpType.is_le, mybir.AluOpType.mult,
                )
                masks[k] = (msk, blo)

            for j in range(nb):
                klo, khi = ranges[j]
                ps = psum_pool.tile([P, D], F32, name=f"ps{j}", tag="ps")
                for k in range(klo, khi):
                    msk, blo = masks[k]
                    # Same 0/1 mask feeds both DoubleRow planes via a
                    # stride-0 middle AP dim.
                    lh = (
                        msk[:, (j - blo) * P : (j - blo + 1) * P]
                        .unsqueeze(1)
                        .broadcast_to((P, 2, P))
                    )
                    cj, cs = k2chunk[k]
                    rh = fhi_tiles[cj]
                    first = k == klo
                    last = k == khi - 1
                    for n0, nn in ((0, 512), (512, 256)):
                        nc.tensor.matmul(
                            ps[:, n0 : n0 + nn],
                            lh,
                            rh[:, cs, :, n0 : n0 + nn],
                            start=first,
                            stop=last,
                            perf_mode=DR,
                        )
                os = out_pool.tile([P, D], FP16, name=f"os{j}", tag="os")
                nc.scalar.mul(out=os[:], in_=ps[:], mul=iv_sb[:, j : j + 1])
                # Outputs on the SP ring so the ACT sequencer never stalls
                # between evacuation copies.
                nc.sync.dma_start(out=out_r[:, j, :], in_=os[:])

    nc.finalize()
    return nc


def _prepare(features, begins, ends):
    feats = np.asarray(features, dtype=np.float32)
    assert feats.shape == (B, T, D), feats.shape
    b = np.clip(np.asarray(begins).astype(np.int64), 0, T - 1)
    e = np.asarray(ends).astype(np.int64)
    # Reference gathers at most MAXWIN tokens starting at b; empty -> count 1.
    e_eff = np.clip(e, b, np.minimum(b + MAXWIN, T))
    counts = np.maximum(e_eff - b, 1).astype(np.float32)
    inv = (1.0 / counts).astype(np.float32)

    ranges, perms = _assign(b, e_eff)
    nb = len(ranges)
    Wp = nb * P

    # fp8 A/R planes: A = fp8(x), R = fp8(x - A); shuffle to [P, NKT, 2, D]
    # (partition p holds tokens {p, 128+p, ...}, planes contiguous per tile)
    f8 = mybir.dt.np(FP8)
    A = feats.astype(f8)
    R = (feats - A.astype(np.float32)).astype(f8)
    hi = np.ascontiguousarray(
        np.stack(
            [A.reshape(B, NKT, P, D), R.reshape(B, NKT, P, D)], axis=3
        ).transpose(0, 2, 1, 3, 4)
    )  # [B, P, NKT, 2, D]

    iota = (
        np.arange(NKT)[None, :] * P + np.arange(P)[:, None] - 2048
    ).astype(np.float32)
    in_maps = []
    idx_maps = []
    for c in range(B):
        idx = np.array(
            [w for blk in perms[c] for w in blk], dtype=np.int64
        )  # [Wp], -1 = pad
        used = idx >= 0
        bp = np.full(Wp, 2047 + 2048, np.int64)  # pad: begin beyond any token
        ep = np.zeros(Wp, np.int64)  # pad: end before any token
        bp[used] = b[c][idx[used]]
        ep[used] = e_eff[c][idx[used]]
        metac = np.ascontiguousarray(
            (np.stack([bp, ep]) - 2048).astype(np.float16).reshape(1, 2, Wp)
        )
        invc = np.ones(Wp, np.float32)
        invc[used] = inv[c][idx[used]]
        ioiv_c = np.zeros((P, P), np.float32)
        ioiv_c[:, 0:NKT] = iota
        ioiv_c[:, NKT : NKT + nb] = invc.reshape(nb, P).T
        in_maps.append({"fhi": hi[c], "meta": metac, "ioiv": ioiv_c})
        idx_maps.append(idx)
    return ranges, in_maps, idx_maps


def run(features, begins, ends, trace=False):
    """Build + run on 8 NeuronCores; returns (output, BassKernelResults)."""
    ranges, in_maps, idx_maps = _prepare(features, begins, ends)
    nc = _build_program(ranges)
    res = run_bass_kernel_spmd(nc, in_maps, list(range(B)), trace=trace)
    out = np.zeros((B, W, D), np.float32)
    for c in range(B):
        idx = idx_maps[c]
        used = idx >= 0
        dev = res.results[c]["out"].astype(np.float32)
        out[c, idx[used]] = dev[used]
    return out, res


def kernel(features, begins, ends):
    out, _ = run(features, begins, ends, trace=False)
    return out
